# revision 5
# baseline (speedup 1.0000x reference)
"""Trainium2 Bass kernel for GroupNorm + single-head spatial self-attention
(diffusion-style attention block).

Computation (per image):
    n  = GroupNorm(x; 32 groups) * gn_scale + gn_bias          [C, N]
    q  = wq @ n + bq ; k = wk @ n + bk ; v = wv @ n + bv
    A  = softmax(q^T k / sqrt(C), axis over keys)
    out = x + wp @ (A @ v)^T + bp

Shapes: B=32, C=512, H=W=32 (N = H*W = 1024 spatial positions).

Sharding: data-parallel over batch — 4 images per core on 8 cores, all
weights replicated, no cross-core communication.

Layout strategy (per image, all on-chip after the x load):
  - n, q, k are kept channel-major [C, N] (4 tiles of [128, 1024]).
  - S^T = k^T q is computed directly in [keys m, queries n] layout so
    that softmax normalization can be DEFERRED: E = exp(S^T) feeds both
    the AV matmul (as rhs) and the denominator row (ones^T @ E via PE).
  - v is computed position-major [N, C] (lhsT = n chunks, rhs = wv^T),
    so AV (lhsT = v chunks, rhs = E) yields attn-out channel-major
    [C, N] with NO transposes anywhere.
  - The softmax denominator reciprocal r[n] is broadcast to [128, n]
    with a rank-1 PE outer product and applied at the very end:
    y = x + (wp @ raw)*r + bp.
All big matmuls run as float32r (full-rate fp32 on the PE at free-dim
512) accumulating in fp32 PSUM.
"""

import numpy as np

import concourse.bacc as bacc
import concourse.tile as tile
from concourse import mybir
from concourse import bass_utils

F32 = mybir.dt.float32
F32R = mybir.dt.float32r
AX = mybir.AxisListType.X
OP = mybir.AluOpType
AF = mybir.ActivationFunctionType

B, C, H, W = 32, 512, 32, 32
HW = H * W                      # 1024 spatial positions
NCORES = 8
BPC = B // NCORES               # images per core
G = 32                          # groups
GS = C // G                     # channels per group
EPS = 1e-5
P = 128
NCH = C // P                    # 4 channel chunks of 128
NPT = HW // P                   # 8 position tiles of 128
SCALE = float(C) ** -0.5


def _build():
    nc = bacc.Bacc("TRN2", target_bir_lowering=False, debug=False)

    xs = nc.dram_tensor("xs", [BPC, C, HW], F32, kind="ExternalInput")
    wqT = nc.dram_tensor("wqT", [C, C], F32R, kind="ExternalInput")
    wkT = nc.dram_tensor("wkT", [C, C], F32R, kind="ExternalInput")
    wvT = nc.dram_tensor("wvT", [C, C], F32R, kind="ExternalInput")
    wpT = nc.dram_tensor("wpT", [C, C], F32R, kind="ExternalInput")
    # bias pack columns: 0=bq 1=bk 2=bp 3=gn_scale 4=gn_bias
    biasp = nc.dram_tensor("biasp", [NCH, P, 5], F32, kind="ExternalInput")
    bvb = nc.dram_tensor("bvb", [P, C], F32, kind="ExternalInput")
    gmask = nc.dram_tensor("gmask", [NCH, P, G], F32, kind="ExternalInput")
    gmaskT = nc.dram_tensor("gmaskT", [P, C], F32, kind="ExternalInput")
    onesd = nc.dram_tensor("onesd", [P, P], F32R, kind="ExternalInput")
    ys = nc.dram_tensor("ys", [BPC, C, HW], F32, kind="ExternalOutput")

    xs_ap, ys_ap = xs.ap(), ys.ap()

    with tile.TileContext(nc) as tc:
        with (
            tc.tile_pool(name="consts", bufs=1) as cp,
            tc.tile_pool(name="work", bufs=1) as wpool,
            tc.tile_pool(name="psum", bufs=2, space="PSUM") as pp,
        ):
            # ---- constants (loaded once) ----
            def const_mat(dram, tagbase):
                tiles = []
                for c in range(NCH):
                    t = cp.tile([P, C], F32R, tag=f"{tagbase}{c}", name=f"{tagbase}{c}")
                    nc.sync.dma_start(out=t, in_=dram.ap()[c * P:(c + 1) * P, :])
                    tiles.append(t)
                return tiles

            wq_sb = const_mat(wqT, "wq")
            wk_sb = const_mat(wkT, "wk")
            wv_sb = const_mat(wvT, "wv")
            wp_sb = const_mat(wpT, "wp")

            gm_sb = []
            for c in range(NCH):
                t = cp.tile([P, G], F32, tag=f"gm{c}", name=f"gm{c}")
                nc.sync.dma_start(out=t, in_=gmask.ap()[c])
                gm_sb.append(t)
            gmT_sb = cp.tile([P, C], F32, tag="gmT", name="gmT")
            nc.sync.dma_start(out=gmT_sb, in_=gmaskT.ap())

            bias_sb = []
            for c in range(NCH):
                t = cp.tile([P, 5], F32, tag=f"bias{c}", name=f"bias{c}")
                nc.sync.dma_start(out=t, in_=biasp.ap()[c])
                bias_sb.append(t)
            bvb_sb = cp.tile([P, C], F32, tag="bvb", name="bvb")
            nc.sync.dma_start(out=bvb_sb, in_=bvb.ap())

            ones_col = cp.tile([P, 1], F32R, tag="ones_col", name="ones_col")
            nc.sync.dma_start(out=ones_col, in_=onesd.ap()[:, 0:1])
            ones_row = cp.tile([1, P], F32R, tag="ones_row", name="ones_row")
            nc.sync.dma_start(out=ones_row, in_=onesd.ap()[0:1, :])
            eps_sb = cp.tile([P, 1], F32, tag="eps", name="eps")
            nc.vector.memset(eps_sb, EPS)
            zero_col = cp.tile([P, 1], F32, tag="zero", name="zero")
            nc.vector.memset(zero_col, 0.0)

            HWH = HW // 2  # half of the free dim, 512

            for b in range(BPC):
                # ---- load x + GroupNorm statistics ----
                x_sb = []
                for c in range(NCH):
                    xt = wpool.tile([P, HW], F32, tag=f"x{c}", bufs=2,
                                    name=f"x_b{b}_{c}")
                    nc.sync.dma_start(out=xt, in_=xs_ap[b, c * P:(c + 1) * P, :])
                    x_sb.append(xt)

                st = []
                for c in range(NCH):
                    s = wpool.tile([P, 2], F32, tag=f"st{c}", name=f"st_b{b}_{c}")
                    nc.vector.reduce_sum(out=s[:, 0:1], in_=x_sb[c], axis=AX)
                    scr = wpool.tile([P, HW], F32, tag="sqscr", bufs=2,
                                     name=f"sqscr_b{b}_{c}")
                    nc.scalar.activation(out=scr, in_=x_sb[c], func=AF.Square,
                                         bias=zero_col, accum_out=s[:, 1:2])
                    st.append(s)

                gp = pp.tile([G, 2], F32, tag="sp", name=f"gp_b{b}")
                for c in range(NCH):
                    nc.tensor.matmul(gp, lhsT=gm_sb[c], rhs=st[c],
                                     start=(c == 0), stop=(c == NCH - 1))

                # gmr: col0 = group mean, col1 = group rstd (rows >= G zero)
                gmr = wpool.tile([P, 2], F32, tag="gmr", name=f"gmr_b{b}")
                nc.vector.memset(gmr, 0.0)
                nc.vector.tensor_scalar(gmr[:G, 0:1], gp[:G, 0:1],
                                        1.0 / (GS * HW), None, OP.mult)
                e2 = wpool.tile([P, 1], F32, tag="e2", name=f"e2_b{b}")
                nc.vector.tensor_scalar(e2[:G], gp[:G, 1:2],
                                        1.0 / (GS * HW), None, OP.mult)
                m2 = wpool.tile([P, 1], F32, tag="m2", name=f"m2_b{b}")
                nc.vector.tensor_mul(m2[:G], gmr[:G, 0:1], gmr[:G, 0:1])
                var = wpool.tile([P, 1], F32, tag="var", name=f"var_b{b}")
                nc.vector.tensor_sub(var[:G], e2[:G], m2[:G])
                sd = wpool.tile([P, 1], F32, tag="sd", name=f"sd_b{b}")
                nc.scalar.activation(out=sd[:G], in_=var[:G], func=AF.Sqrt,
                                     bias=eps_sb[:G])
                nc.vector.reciprocal(out=gmr[:G, 1:2], in_=sd[:G])

                # ---- normalize: n = x * a + bb (per-channel a, bb) ----
                n_sb = []
                for c in range(NCH):
                    bc = pp.tile([P, 2], F32, tag="sp", name=f"bc_b{b}_{c}")
                    nc.tensor.matmul(bc, lhsT=gmT_sb[:, c * P:(c + 1) * P],
                                     rhs=gmr, start=True, stop=True)
                    a = wpool.tile([P, 1], F32, tag=f"a{c}", name=f"a_b{b}_{c}")
                    nc.vector.tensor_mul(a, bc[:, 1:2], bias_sb[c][:, 3:4])
                    gt = wpool.tile([P, 1], F32, tag=f"gt{c}", name=f"gt_b{b}_{c}")
                    nc.vector.tensor_mul(gt, bc[:, 0:1], a)
                    bb = wpool.tile([P, 1], F32, tag=f"bb{c}", name=f"bb_b{b}_{c}")
                    nc.vector.tensor_sub(bb, bias_sb[c][:, 4:5], gt)
                    nt = wpool.tile([P, HW], F32R, tag=f"n{c}", name=f"n_b{b}_{c}")
                    nc.vector.tensor_scalar(nt, x_sb[c], a, bb, OP.mult, OP.add)
                    n_sb.append(nt)

                # ---- q, k projections (channel-major [C, HW]) ----
                qk_sb = []
                for (w_t, bcol, tagbase) in ((wq_sb, 0, "q"), (wk_sb, 1, "k")):
                    dst = []
                    for o in range(NCH):
                        acc = pp.tile([P, HW], F32, tag="acc2",
                                      name=f"{tagbase}acc_b{b}_{o}")
                        for c in range(NCH):
                            for h in range(2):
                                nc.tensor.matmul(
                                    acc[:, h * HWH:(h + 1) * HWH],
                                    lhsT=(w_t[c][:, o * P:(o + 1) * P]),
                                    rhs=(n_sb[c][:, h * HWH:(h + 1) * HWH]),
                                    start=(c == 0), stop=(c == NCH - 1))
                        t = wpool.tile([P, HW], F32R, tag=f"{tagbase}{o}",
                                       name=f"{tagbase}_b{b}_{o}")
                        nc.vector.tensor_scalar(t, acc, bias_sb[o][:, bcol:bcol + 1],
                                                None, OP.add)
                        dst.append(t)
                    qk_sb.append(dst)
                q_sb, k_sb = qk_sb

                # ---- v projection (position-major [HW, C]) ----
                v_sb = []
                for t8 in range(NPT):
                    acc = pp.tile([P, HWH], F32, tag="acc1", name=f"vacc_b{b}_{t8}")
                    for c in range(NCH):
                        nc.tensor.matmul(acc,
                                         lhsT=(n_sb[c][:, t8 * P:(t8 + 1) * P]),
                                         rhs=(wv_sb[c]),
                                         start=(c == 0), stop=(c == NCH - 1))
                    vt = wpool.tile([P, HWH], F32R, tag=f"v{t8}", name=f"v_b{b}_{t8}")
                    nc.vector.tensor_add(vt, acc, bvb_sb)
                    v_sb.append(vt)

                # ---- S^T = k^T q, E = exp(S^T * C^-0.5) ----
                e_sb = []
                for m in range(NPT):
                    acc = pp.tile([P, HW], F32, tag="acc2", name=f"sacc_b{b}_{m}")
                    for c in range(NCH):
                        for h in range(2):
                            nc.tensor.matmul(
                                acc[:, h * HWH:(h + 1) * HWH],
                                lhsT=(k_sb[c][:, m * P:(m + 1) * P]),
                                rhs=(q_sb[c][:, h * HWH:(h + 1) * HWH]),
                                start=(c == 0), stop=(c == NCH - 1))
                    et = wpool.tile([P, HW], F32R, tag=f"e{m}", name=f"e_b{b}_{m}")
                    nc.scalar.activation(out=et, in_=acc, func=AF.Exp,
                                         bias=zero_col, scale=SCALE)
                    e_sb.append(et)

                # ---- softmax denominator row + reciprocal broadcast ----
                rd = wpool.tile([1, HW], F32R, tag="rd", name=f"rd_b{b}")
                for h in range(2):
                    dn = pp.tile([1, HWH], F32, tag="sp", name=f"dn_b{b}_{h}")
                    for m in range(NPT):
                        nc.tensor.matmul(dn, lhsT=(ones_col),
                                         rhs=(e_sb[m][:, h * HWH:(h + 1) * HWH]),
                                         start=(m == 0), stop=(m == NPT - 1))
                    with nc.allow_low_precision(reason="fp32r feed for PE bcast"):
                        nc.vector.reciprocal(out=rd[:, h * HWH:(h + 1) * HWH],
                                             in_=dn)
                racc = pp.tile([P, HW], F32, tag="acc2", name=f"racc_b{b}")
                for h in range(2):
                    nc.tensor.matmul(racc[:, h * HWH:(h + 1) * HWH],
                                     lhsT=(ones_row),
                                     rhs=(rd[:1, h * HWH:(h + 1) * HWH]),
                                     start=True, stop=True)
                r_sb = wpool.tile([P, HW], F32, tag="k0", name=f"r_b{b}")
                nc.vector.tensor_copy(out=r_sb, in_=racc)

                # ---- attn-out = v^T E  (channel-major [C, HW]) ----
                o_sb = []
                for ct in range(NCH):
                    acc = pp.tile([P, HW], F32, tag="acc2", name=f"oacc_b{b}_{ct}")
                    for m in range(NPT):
                        for h in range(2):
                            nc.tensor.matmul(
                                acc[:, h * HWH:(h + 1) * HWH],
                                lhsT=(v_sb[m][:, ct * P:(ct + 1) * P]),
                                rhs=(e_sb[m][:, h * HWH:(h + 1) * HWH]),
                                start=(m == 0), stop=(m == NPT - 1))
                    ot = wpool.tile([P, HW], F32R, tag=f"q{ct}", name=f"o_b{b}_{ct}")
                    nc.vector.tensor_copy(out=ot, in_=acc)
                    o_sb.append(ot)

                # ---- projection + deferred softmax scale + residual ----
                for o in range(NCH):
                    acc = pp.tile([P, HW], F32, tag="acc2", name=f"pacc_b{b}_{o}")
                    for c in range(NCH):
                        for h in range(2):
                            nc.tensor.matmul(
                                acc[:, h * HWH:(h + 1) * HWH],
                                lhsT=(wp_sb[c][:, o * P:(o + 1) * P]),
                                rhs=(o_sb[c][:, h * HWH:(h + 1) * HWH]),
                                start=(c == 0), stop=(c == NCH - 1))
                    t1 = wpool.tile([P, HW], F32, tag="k1", name=f"t1_b{b}_{o}")
                    nc.vector.tensor_mul(t1, acc, r_sb)
                    yt = wpool.tile([P, HW], F32, tag=f"n{o}", name=f"y_b{b}_{o}")
                    nc.vector.scalar_tensor_tensor(
                        out=yt, in0=t1, scalar=bias_sb[o][:, 2:3], in1=x_sb[o],
                        op0=OP.add, op1=OP.add)
                    nc.sync.dma_start(out=ys_ap[b, o * P:(o + 1) * P, :], in_=yt)

    nc.compile()
    return nc


_NC = None


def _get_nc():
    global _NC
    if _NC is None:
        _NC = _build()
    return _NC


def _host_inputs(x, gn_scale, gn_bias, wq, bq, wk, bk, wv, bv, wp, bp):
    x = np.ascontiguousarray(np.asarray(x, np.float32).reshape(B, C, HW))
    f = lambda t: np.ascontiguousarray(np.asarray(t, np.float32))
    gn_scale, gn_bias = f(gn_scale), f(gn_bias)
    bq, bk, bv, bp = f(bq), f(bk), f(bv), f(bp)
    wq, wk, wv, wp = f(wq), f(wk), f(wv), f(wp)

    biasp = np.stack([bq, bk, bp, gn_scale, gn_bias], 1).reshape(NCH, P, 5)
    biasp = np.ascontiguousarray(biasp)
    bvb = np.ascontiguousarray(np.tile(bv[None, :], (P, 1)))
    ch = np.arange(C)
    gmask_full = (ch[:, None] // GS == np.arange(G)[None, :]).astype(np.float32)
    gmask = np.ascontiguousarray(gmask_full.reshape(NCH, P, G))
    gmaskT = np.zeros((P, C), np.float32)
    gmaskT[:G, :] = gmask_full.T
    common = {
        "wqT": np.ascontiguousarray(wq.T),
        "wkT": np.ascontiguousarray(wk.T),
        "wvT": np.ascontiguousarray(wv.T),
        "wpT": np.ascontiguousarray(wp.T),
        "biasp": biasp,
        "bvb": bvb,
        "gmask": gmask,
        "gmaskT": gmaskT,
        "onesd": np.ones((P, P), np.float32),
    }
    in_maps = []
    for i in range(NCORES):
        m = dict(common)
        m["xs"] = np.ascontiguousarray(x[i * BPC:(i + 1) * BPC])
        in_maps.append(m)
    return in_maps


def _run(in_maps, trace=False):
    nc = _get_nc()
    return bass_utils.run_bass_kernel_spmd(nc, in_maps, list(range(NCORES)),
                                           trace=trace)


def kernel(**inputs):
    in_maps = _host_inputs(**inputs)
    res = _run(in_maps, trace=False)
    y = np.concatenate([r["ys"] for r in res.results], axis=0)
    return y.reshape(B, C, H, W)


def run_traced(**inputs):
    """Like kernel() but with NTFF tracing; returns (y, exec_time_ns)."""
    in_maps = _host_inputs(**inputs)
    res = _run(in_maps, trace=True)
    y = np.concatenate([r["ys"] for r in res.results], axis=0)
    return y.reshape(B, C, H, W), res.exec_time_ns


# revision 6
# speedup vs baseline: 1.1064x; 1.1064x over previous
"""Trainium2 Bass kernel for GroupNorm + single-head spatial self-attention
(diffusion-style attention block).

Computation (per image):
    n  = GroupNorm(x; 32 groups) * gn_scale + gn_bias          [C, N]
    q  = wq @ n + bq ; k = wk @ n + bk ; v = wv @ n + bv
    A  = softmax(q^T k / sqrt(C), axis over keys)
    out = x + wp @ (A @ v)^T + bp

Shapes: B=32, C=512, H=W=32 (N = H*W = 1024 spatial positions).

Sharding: data-parallel over batch — 4 images per core on 8 cores, all
weights replicated, no cross-core communication.

Layout strategy (per image, all on-chip after the x load):
  - n, q, k are kept channel-major [C, N] (4 tiles of [128, 1024]).
  - S^T = k^T q is computed directly in [keys m, queries n] layout so
    that softmax normalization can be DEFERRED: E = exp(S^T) feeds both
    the AV matmul (as rhs) and the denominator row (ones^T @ E via PE).
  - v is computed position-major [N, C] (lhsT = n chunks, rhs = wv^T),
    so AV (lhsT = v chunks, rhs = E) yields attn-out channel-major
    [C, N] with NO transposes anywhere.
  - The softmax denominator reciprocal r[n] is broadcast to [128, n]
    with a rank-1 PE outer product and applied at the very end:
    y = x + (wp @ raw)*r + bp.
Bias algebra: bk shifts every score in a softmax column by the same
amount, so it cancels exactly — never applied. bv passes through
attention into a constant channel offset wp@bv, folded into bp on the
host. Only bq is applied on-device.
All big matmuls run as float32r (full-rate fp32 on the PE at free-dim
512) accumulating in fp32 PSUM. Emission is software-pipelined one
image ahead so GroupNorm/DVE work for image b+1 hides under image b's
attention matmuls.
"""

import numpy as np

import concourse.bacc as bacc
import concourse.tile as tile
from concourse import mybir
from concourse import bass_utils

F32 = mybir.dt.float32
F32R = mybir.dt.float32r
AX = mybir.AxisListType.X
OP = mybir.AluOpType
AF = mybir.ActivationFunctionType

B, C, H, W = 32, 512, 32, 32
HW = H * W                      # 1024 spatial positions
HWH = HW // 2                   # 512 = max fp32 matmul free dim
NCORES = 8
BPC = B // NCORES               # images per core
G = 32                          # groups
GS = C // G                     # channels per group
EPS = 1e-5
P = 128
NCH = C // P                    # 4 channel chunks of 128
NPT = HW // P                   # 8 position tiles of 128
SCALE = float(C) ** -0.5


def _build():
    nc = bacc.Bacc("TRN2", target_bir_lowering=False, debug=False)

    xs = nc.dram_tensor("xs", [BPC, C, HW], F32, kind="ExternalInput")
    wqT = nc.dram_tensor("wqT", [C, C], F32R, kind="ExternalInput")
    wkT = nc.dram_tensor("wkT", [C, C], F32R, kind="ExternalInput")
    wvT = nc.dram_tensor("wvT", [C, C], F32R, kind="ExternalInput")
    wpT = nc.dram_tensor("wpT", [C, C], F32R, kind="ExternalInput")
    # bias pack columns: 0=bq 1=bp' (=bp+wp@bv) 2=gn_scale 3=gn_bias
    biasp = nc.dram_tensor("biasp", [NCH, P, 4], F32, kind="ExternalInput")
    gmask = nc.dram_tensor("gmask", [NCH, P, G], F32, kind="ExternalInput")
    gmaskT = nc.dram_tensor("gmaskT", [P, C], F32, kind="ExternalInput")
    onesd = nc.dram_tensor("onesd", [P, P], F32R, kind="ExternalInput")
    ys = nc.dram_tensor("ys", [BPC, C, HW], F32, kind="ExternalOutput")

    xs_ap, ys_ap = xs.ap(), ys.ap()

    with tile.TileContext(nc) as tc:
        with (
            tc.tile_pool(name="consts", bufs=1) as cp,
            tc.tile_pool(name="work", bufs=1) as wpool,
            tc.tile_pool(name="psum", bufs=2, space="PSUM") as pp,
        ):
            st_ = {}   # mutable per-image state keyed (name, b)

            # ---- image-0 x load first so GN starts before weights land ----
            def load_x(b):
                tiles = []
                for c in range(NCH):
                    xt = wpool.tile([P, HW], F32, tag=f"x{c}", bufs=2,
                                    name=f"x_b{b}_{c}")
                    nc.sync.dma_start(out=xt, in_=xs_ap[b, c * P:(c + 1) * P, :])
                    tiles.append(xt)
                st_["x", b] = tiles

            load_x(0)

            # ---- constants ----
            def const_mat(dram, tagbase):
                tiles = []
                for c in range(NCH):
                    t = cp.tile([P, C], F32R, tag=f"{tagbase}{c}",
                                name=f"{tagbase}{c}")
                    nc.sync.dma_start(out=t, in_=dram.ap()[c * P:(c + 1) * P, :])
                    tiles.append(t)
                return tiles

            gm_sb = []
            for c in range(NCH):
                t = cp.tile([P, G], F32, tag=f"gm{c}", name=f"gm{c}")
                nc.sync.dma_start(out=t, in_=gmask.ap()[c])
                gm_sb.append(t)
            gmT_sb = cp.tile([P, C], F32, tag="gmT", name="gmT")
            nc.sync.dma_start(out=gmT_sb, in_=gmaskT.ap())
            bias_sb = []
            for c in range(NCH):
                t = cp.tile([P, 4], F32, tag=f"bias{c}", name=f"bias{c}")
                nc.sync.dma_start(out=t, in_=biasp.ap()[c])
                bias_sb.append(t)
            eps_sb = cp.tile([P, 1], F32, tag="eps", name="eps")
            nc.vector.memset(eps_sb, EPS)
            zero_col = cp.tile([P, 1], F32, tag="zero", name="zero")
            nc.vector.memset(zero_col, 0.0)

            wq_sb = const_mat(wqT, "wq")
            wk_sb = const_mat(wkT, "wk")
            wv_sb = const_mat(wvT, "wv")
            wp_sb = const_mat(wpT, "wp")
            ones_col = cp.tile([P, 1], F32R, tag="ones_col", name="ones_col")
            nc.sync.dma_start(out=ones_col, in_=onesd.ap()[:, 0:1])
            ones_row = cp.tile([1, P], F32R, tag="ones_row", name="ones_row")
            nc.sync.dma_start(out=ones_row, in_=onesd.ap()[0:1, :])

            # ---- per-image phases ----
            def gn_stats(b):
                x_sb = st_["x", b]
                stt = []
                for c in range(NCH):
                    s = wpool.tile([P, 2], F32, tag=f"st{c}", name=f"st_b{b}_{c}")
                    nc.vector.reduce_sum(out=s[:, 0:1], in_=x_sb[c], axis=AX)
                    scr = wpool.tile([P, HW], F32, tag="sqscr", bufs=2,
                                     name=f"sqscr_b{b}_{c}")
                    nc.scalar.activation(out=scr, in_=x_sb[c], func=AF.Square,
                                         bias=zero_col, accum_out=s[:, 1:2])
                    stt.append(s)

                gp = pp.tile([G, 2], F32, tag="sp", name=f"gp_b{b}")
                for c in range(NCH):
                    nc.tensor.matmul(gp, lhsT=gm_sb[c], rhs=stt[c],
                                     start=(c == 0), stop=(c == NCH - 1))

                # gmr: col0 = group mean, col1 = group rstd (rows >= G zero)
                gmr = wpool.tile([P, 2], F32, tag="gmr", name=f"gmr_b{b}")
                nc.vector.memset(gmr, 0.0)
                nc.vector.tensor_scalar(gmr[:G, 0:1], gp[:G, 0:1],
                                        1.0 / (GS * HW), None, OP.mult)
                e2 = wpool.tile([P, 1], F32, tag="e2", name=f"e2_b{b}")
                nc.vector.tensor_scalar(e2[:G], gp[:G, 1:2],
                                        1.0 / (GS * HW), None, OP.mult)
                m2 = wpool.tile([P, 1], F32, tag="m2", name=f"m2_b{b}")
                nc.vector.tensor_mul(m2[:G], gmr[:G, 0:1], gmr[:G, 0:1])
                var = wpool.tile([P, 1], F32, tag="var", name=f"var_b{b}")
                nc.vector.tensor_sub(var[:G], e2[:G], m2[:G])
                sd = wpool.tile([P, 1], F32, tag="sd", name=f"sd_b{b}")
                nc.scalar.activation(out=sd[:G], in_=var[:G], func=AF.Sqrt,
                                     bias=eps_sb[:G])
                nc.vector.reciprocal(out=gmr[:G, 1:2], in_=sd[:G])
                st_["gmr", b] = gmr

            def normalize(b):
                x_sb, gmr = st_["x", b], st_.pop(("gmr", b))
                n_sb = []
                for c in range(NCH):
                    bc = pp.tile([P, 2], F32, tag="sp", name=f"bc_b{b}_{c}")
                    nc.tensor.matmul(bc, lhsT=gmT_sb[:, c * P:(c + 1) * P],
                                     rhs=gmr, start=True, stop=True)
                    a = wpool.tile([P, 1], F32, tag=f"a{c}", name=f"a_b{b}_{c}")
                    nc.vector.tensor_mul(a, bc[:, 1:2], bias_sb[c][:, 2:3])
                    gt = wpool.tile([P, 1], F32, tag=f"gt{c}", name=f"gt_b{b}_{c}")
                    nc.vector.tensor_mul(gt, bc[:, 0:1], a)
                    bb = wpool.tile([P, 1], F32, tag=f"bb{c}", name=f"bb_b{b}_{c}")
                    nc.vector.tensor_sub(bb, bias_sb[c][:, 3:4], gt)
                    nt = wpool.tile([P, HW], F32R, tag=f"n{c}", name=f"n_b{b}_{c}")
                    nc.vector.tensor_scalar(nt, x_sb[c], a, bb, OP.mult, OP.add)
                    n_sb.append(nt)
                st_["n", b] = n_sb

            def qkv(b):
                n_sb = st_.pop(("n", b))
                # q: + bq (DVE per-partition add); k: bias cancels -> ACT copy
                for (w_t, tagbase) in ((wq_sb, "q"), (wk_sb, "k")):
                    dst = []
                    for o in range(NCH):
                        acc = pp.tile([P, HW], F32, tag="acc2",
                                      name=f"{tagbase}acc_b{b}_{o}")
                        for c in range(NCH):
                            for h in range(2):
                                nc.tensor.matmul(
                                    acc[:, h * HWH:(h + 1) * HWH],
                                    lhsT=w_t[c][:, o * P:(o + 1) * P],
                                    rhs=n_sb[c][:, h * HWH:(h + 1) * HWH],
                                    start=(c == 0), stop=(c == NCH - 1))
                        t = wpool.tile([P, HW], F32R, tag=f"{tagbase}{o}",
                                       name=f"{tagbase}_b{b}_{o}")
                        if tagbase == "q":
                            nc.vector.tensor_scalar(t, acc,
                                                    bias_sb[o][:, 0:1],
                                                    None, OP.add)
                        else:
                            nc.scalar.copy(t, acc)
                        dst.append(t)
                    st_[tagbase, b] = dst
                v_sb = []
                for t8 in range(NPT):
                    acc = pp.tile([P, HWH], F32, tag="acc1", name=f"vacc_b{b}_{t8}")
                    for c in range(NCH):
                        nc.tensor.matmul(acc,
                                         lhsT=n_sb[c][:, t8 * P:(t8 + 1) * P],
                                         rhs=wv_sb[c],
                                         start=(c == 0), stop=(c == NCH - 1))
                    vt = wpool.tile([P, HWH], F32R, tag=f"v{t8}",
                                    name=f"v_b{b}_{t8}")
                    nc.scalar.copy(vt, acc)
                    v_sb.append(vt)
                st_["v", b] = v_sb

            def st_phase(b):
                q_sb, k_sb = st_.pop(("q", b)), st_.pop(("k", b))
                e_sb = []
                for m in range(NPT):
                    acc = pp.tile([P, HW], F32, tag="acc2", name=f"sacc_b{b}_{m}")
                    for c in range(NCH):
                        for h in range(2):
                            nc.tensor.matmul(
                                acc[:, h * HWH:(h + 1) * HWH],
                                lhsT=k_sb[c][:, m * P:(m + 1) * P],
                                rhs=q_sb[c][:, h * HWH:(h + 1) * HWH],
                                start=(c == 0), stop=(c == NCH - 1))
                    et = wpool.tile([P, HW], F32R, tag=f"e{m}", name=f"e_b{b}_{m}")
                    nc.scalar.activation(out=et, in_=acc, func=AF.Exp,
                                         bias=zero_col, scale=SCALE)
                    e_sb.append(et)
                st_["e", b] = e_sb

            def av_den(b):
                e_sb, v_sb = st_["e", b], st_.pop(("v", b))
                o_sb = []
                for ct in range(NCH):
                    acc = pp.tile([P, HW], F32, tag="acc2", name=f"oacc_b{b}_{ct}")
                    for m in range(NPT):
                        for h in range(2):
                            nc.tensor.matmul(
                                acc[:, h * HWH:(h + 1) * HWH],
                                lhsT=v_sb[m][:, ct * P:(ct + 1) * P],
                                rhs=e_sb[m][:, h * HWH:(h + 1) * HWH],
                                start=(m == 0), stop=(m == NPT - 1))
                    ot = wpool.tile([P, HW], F32R, tag=f"q{ct}", name=f"o_b{b}_{ct}")
                    nc.scalar.copy(ot, acc)
                    o_sb.append(ot)
                st_["o", b] = o_sb

                rd = wpool.tile([1, HW], F32R, tag="rd", name=f"rd_b{b}")
                for h in range(2):
                    dn = pp.tile([1, HWH], F32, tag="sp", name=f"dn_b{b}_{h}")
                    for m in range(NPT):
                        nc.tensor.matmul(dn, lhsT=ones_col,
                                         rhs=e_sb[m][:, h * HWH:(h + 1) * HWH],
                                         start=(m == 0), stop=(m == NPT - 1))
                    with nc.allow_low_precision(reason="fp32r feed for PE bcast"):
                        nc.vector.reciprocal(out=rd[:, h * HWH:(h + 1) * HWH],
                                             in_=dn)
                racc = pp.tile([P, HW], F32, tag="acc2", name=f"racc_b{b}")
                for h in range(2):
                    nc.tensor.matmul(racc[:, h * HWH:(h + 1) * HWH],
                                     lhsT=ones_row,
                                     rhs=rd[:1, h * HWH:(h + 1) * HWH],
                                     start=True, stop=True)
                r_sb = wpool.tile([P, HW], F32, tag="r", name=f"r_b{b}")
                nc.scalar.copy(r_sb, racc)
                st_.pop(("e", b))
                st_["r", b] = r_sb

            def proj(b):
                o_sb, r_sb = st_.pop(("o", b)), st_.pop(("r", b))
                x_sb = st_.pop(("x", b))
                for o in range(NCH):
                    acc = pp.tile([P, HW], F32, tag="acc2", name=f"pacc_b{b}_{o}")
                    for c in range(NCH):
                        for h in range(2):
                            nc.tensor.matmul(
                                acc[:, h * HWH:(h + 1) * HWH],
                                lhsT=wp_sb[c][:, o * P:(o + 1) * P],
                                rhs=o_sb[c][:, h * HWH:(h + 1) * HWH],
                                start=(c == 0), stop=(c == NCH - 1))
                    t1 = wpool.tile([P, HW], F32, tag="t1", bufs=2,
                                    name=f"t1_b{b}_{o}")
                    nc.vector.tensor_mul(t1, acc, r_sb)
                    yt = wpool.tile([P, HW], F32, tag=f"y{o}", name=f"y_b{b}_{o}")
                    nc.vector.scalar_tensor_tensor(
                        out=yt, in0=t1, scalar=bias_sb[o][:, 1:2], in1=x_sb[o],
                        op0=OP.add, op1=OP.add)
                    nc.sync.dma_start(out=ys_ap[b, o * P:(o + 1) * P, :], in_=yt)

            # ---- software-pipelined emission, one image ahead ----
            gn_stats(0)
            normalize(0)
            qkv(0)
            for b in range(BPC):
                st_phase(b)
                if b + 1 < BPC:
                    load_x(b + 1)
                    gn_stats(b + 1)
                av_den(b)
                if b + 1 < BPC:
                    normalize(b + 1)
                proj(b)
                if b + 1 < BPC:
                    qkv(b + 1)

    nc.compile()
    return nc


_NC = None


def _get_nc():
    global _NC
    if _NC is None:
        _NC = _build()
    return _NC


def _host_inputs(x, gn_scale, gn_bias, wq, bq, wk, bk, wv, bv, wp, bp):
    x = np.ascontiguousarray(np.asarray(x, np.float32).reshape(B, C, HW))
    f = lambda t: np.ascontiguousarray(np.asarray(t, np.float32))
    gn_scale, gn_bias = f(gn_scale), f(gn_bias)
    bq, bv, bp = f(bq), f(bv), f(bp)
    wq, wk, wv, wp = f(wq), f(wk), f(wv), f(wp)

    bp_eff = bp + wp @ bv  # v-bias passes through softmax-averaging intact
    biasp = np.stack([bq, bp_eff, gn_scale, gn_bias], 1).reshape(NCH, P, 4)
    ch = np.arange(C)
    gmask_full = (ch[:, None] // GS == np.arange(G)[None, :]).astype(np.float32)
    gmask = np.ascontiguousarray(gmask_full.reshape(NCH, P, G))
    gmaskT = np.zeros((P, C), np.float32)
    gmaskT[:G, :] = gmask_full.T
    common = {
        "wqT": np.ascontiguousarray(wq.T),
        "wkT": np.ascontiguousarray(wk.T),
        "wvT": np.ascontiguousarray(wv.T),
        "wpT": np.ascontiguousarray(wp.T),
        "biasp": np.ascontiguousarray(biasp),
        "gmask": gmask,
        "gmaskT": gmaskT,
        "onesd": np.ones((P, P), np.float32),
    }
    in_maps = []
    for i in range(NCORES):
        m = dict(common)
        m["xs"] = np.ascontiguousarray(x[i * BPC:(i + 1) * BPC])
        in_maps.append(m)
    return in_maps


def _run(in_maps, trace=False):
    nc = _get_nc()
    return bass_utils.run_bass_kernel_spmd(nc, in_maps, list(range(NCORES)),
                                           trace=trace)


def kernel(**inputs):
    in_maps = _host_inputs(**inputs)
    res = _run(in_maps, trace=False)
    y = np.concatenate([r["ys"] for r in res.results], axis=0)
    return y.reshape(B, C, H, W)


def run_traced(**inputs):
    """Like kernel() but with NTFF tracing; returns (y, exec_time_ns)."""
    in_maps = _host_inputs(**inputs)
    res = _run(in_maps, trace=True)
    y = np.concatenate([r["ys"] for r in res.results], axis=0)
    return y.reshape(B, C, H, W), res.exec_time_ns


# revision 7
# speedup vs baseline: 1.1602x; 1.0485x over previous
"""Trainium2 Bass kernel for GroupNorm + single-head spatial self-attention
(diffusion-style attention block).

Computation (per image):
    n  = GroupNorm(x; 32 groups) * gn_scale + gn_bias          [C, N]
    q  = wq @ n + bq ; k = wk @ n + bk ; v = wv @ n + bv
    A  = softmax(q^T k / sqrt(C), axis over keys)
    out = x + wp @ (A @ v)^T + bp

Shapes: B=32, C=512, H=W=32 (N = H*W = 1024 spatial positions).

Sharding: data-parallel over batch — 4 images per core on 8 cores, all
weights replicated, no cross-core communication.

Layout strategy (per image, all on-chip after the x load):
  - n, q, k are kept channel-major [C, N] (4 tiles of [128, 1024]).
  - S^T = k^T q is computed directly in [keys m, queries n] layout so
    that softmax normalization can be DEFERRED: E = exp(S^T) feeds both
    the AV matmul (as rhs) and the denominator row (ones^T @ E via PE).
  - v is computed position-major [N, C] (lhsT = n chunks, rhs = wv^T),
    so AV (lhsT = v chunks, rhs = E) yields attn-out channel-major
    [C, N] with NO transposes anywhere.
  - The softmax denominator reciprocal r[n] is broadcast to [128, n]
    with a rank-1 PE outer product and applied at the very end:
    y = x + (wp @ raw)*r + bp.
Bias algebra: bk shifts every score in a softmax column by the same
amount, so it cancels exactly — never applied. bv passes through
attention into a constant channel offset wp@bv, folded into bp on the
host. Only bq is applied on-device.
All big matmuls run as float32r (full-rate fp32 on the PE at free-dim
512) accumulating in fp32 PSUM. Emission is software-pipelined one
image ahead so GroupNorm/DVE work for image b+1 hides under image b's
attention matmuls.
"""

import numpy as np

import concourse.bacc as bacc
import concourse.tile as tile
from concourse import mybir
from concourse import bass_utils

F32 = mybir.dt.float32
F32R = mybir.dt.float32r
AX = mybir.AxisListType.X
OP = mybir.AluOpType
AF = mybir.ActivationFunctionType

B, C, H, W = 32, 512, 32, 32
HW = H * W                      # 1024 spatial positions
HWH = HW // 2                   # 512 = max fp32 matmul free dim
NCORES = 8
BPC = B // NCORES               # images per core
G = 32                          # groups
GS = C // G                     # channels per group
EPS = 1e-5
P = 128
NCH = C // P                    # 4 channel chunks of 128
NPT = HW // P                   # 8 position tiles of 128
SCALE = float(C) ** -0.5


def _build():
    nc = bacc.Bacc("TRN2", target_bir_lowering=False, debug=False)

    xs = nc.dram_tensor("xs", [BPC, C, HW], F32, kind="ExternalInput")
    wqT = nc.dram_tensor("wqT", [C, C], F32R, kind="ExternalInput")
    wkT = nc.dram_tensor("wkT", [C, C], F32R, kind="ExternalInput")
    wvT = nc.dram_tensor("wvT", [C, C], F32R, kind="ExternalInput")
    wpT = nc.dram_tensor("wpT", [C, C], F32R, kind="ExternalInput")
    # bias pack columns: 0=bq 1=bp' (=bp+wp@bv) 2=gn_scale 3=gn_bias
    biasp = nc.dram_tensor("biasp", [NCH, P, 4], F32, kind="ExternalInput")
    gmask = nc.dram_tensor("gmask", [NCH, P, G], F32, kind="ExternalInput")
    gmaskT = nc.dram_tensor("gmaskT", [P, C], F32, kind="ExternalInput")
    onesd = nc.dram_tensor("onesd", [P, P], F32R, kind="ExternalInput")
    ys = nc.dram_tensor("ys", [BPC, C, HW], F32, kind="ExternalOutput")

    xs_ap, ys_ap = xs.ap(), ys.ap()

    with tile.TileContext(nc) as tc:
        with (
            tc.tile_pool(name="consts", bufs=1) as cp,
            tc.tile_pool(name="work", bufs=1) as wpool,
            tc.tile_pool(name="psum", bufs=2, space="PSUM") as pp,
        ):
            st_ = {}   # mutable per-image state keyed (name, b)

            # ---- image-0 x load first so GN starts before weights land ----
            def load_x(b):
                tiles = []
                for c in range(NCH):
                    xt = wpool.tile([P, HW], F32, tag=f"x{c}", bufs=2,
                                    name=f"x_b{b}_{c}")
                    eng = nc.sync if c % 2 == 0 else nc.gpsimd
                    eng.dma_start(out=xt, in_=xs_ap[b, c * P:(c + 1) * P, :])
                    tiles.append(xt)
                st_["x", b] = tiles

            load_x(0)

            # ---- constants ----
            def const_mat(dram, tagbase):
                tiles = []
                for c in range(NCH):
                    t = cp.tile([P, C], F32R, tag=f"{tagbase}{c}",
                                name=f"{tagbase}{c}")
                    nc.sync.dma_start(out=t, in_=dram.ap()[c * P:(c + 1) * P, :])
                    tiles.append(t)
                return tiles

            gm_sb = []
            for c in range(NCH):
                t = cp.tile([P, G], F32, tag=f"gm{c}", name=f"gm{c}")
                nc.sync.dma_start(out=t, in_=gmask.ap()[c])
                gm_sb.append(t)
            gmT_sb = cp.tile([P, C], F32, tag="gmT", name="gmT")
            nc.sync.dma_start(out=gmT_sb, in_=gmaskT.ap())
            bias_sb = []
            for c in range(NCH):
                t = cp.tile([P, 4], F32, tag=f"bias{c}", name=f"bias{c}")
                nc.sync.dma_start(out=t, in_=biasp.ap()[c])
                bias_sb.append(t)
            eps_sb = cp.tile([P, 1], F32, tag="eps", name="eps")
            nc.vector.memset(eps_sb, EPS)
            zero_col = cp.tile([P, 1], F32, tag="zero", name="zero")
            nc.vector.memset(zero_col, 0.0)

            wq_sb = const_mat(wqT, "wq")
            wk_sb = const_mat(wkT, "wk")
            wv_sb = const_mat(wvT, "wv")
            wp_sb = const_mat(wpT, "wp")
            ones_col = cp.tile([P, 1], F32R, tag="ones_col", name="ones_col")
            nc.sync.dma_start(out=ones_col, in_=onesd.ap()[:, 0:1])
            ones_row = cp.tile([1, P], F32R, tag="ones_row", name="ones_row")
            nc.sync.dma_start(out=ones_row, in_=onesd.ap()[0:1, :])

            # ---- per-image phases ----
            def gn_stats(b):
                x_sb = st_["x", b]
                stt = []
                for c in range(NCH):
                    s = wpool.tile([P, 2], F32, tag=f"st{c}", name=f"st_b{b}_{c}")
                    nc.vector.reduce_sum(out=s[:, 0:1], in_=x_sb[c], axis=AX)
                    scr = wpool.tile([P, HW], F32, tag="sqscr", bufs=2,
                                     name=f"sqscr_b{b}_{c}")
                    nc.scalar.activation(out=scr, in_=x_sb[c], func=AF.Square,
                                         bias=zero_col, accum_out=s[:, 1:2])
                    stt.append(s)

                gp = pp.tile([G, 2], F32, tag="sp", name=f"gp_b{b}")
                for c in range(NCH):
                    nc.tensor.matmul(gp, lhsT=gm_sb[c], rhs=stt[c],
                                     start=(c == 0), stop=(c == NCH - 1))

                # gmr: col0 = group mean, col1 = group rstd (rows >= G zero)
                gmr = wpool.tile([P, 2], F32, tag="gmr", name=f"gmr_b{b}")
                nc.vector.memset(gmr, 0.0)
                nc.vector.tensor_scalar(gmr[:G, 0:1], gp[:G, 0:1],
                                        1.0 / (GS * HW), None, OP.mult)
                e2 = wpool.tile([P, 1], F32, tag="e2", name=f"e2_b{b}")
                nc.vector.tensor_scalar(e2[:G], gp[:G, 1:2],
                                        1.0 / (GS * HW), None, OP.mult)
                m2 = wpool.tile([P, 1], F32, tag="m2", name=f"m2_b{b}")
                nc.vector.tensor_mul(m2[:G], gmr[:G, 0:1], gmr[:G, 0:1])
                var = wpool.tile([P, 1], F32, tag="var", name=f"var_b{b}")
                nc.vector.tensor_sub(var[:G], e2[:G], m2[:G])
                sd = wpool.tile([P, 1], F32, tag="sd", name=f"sd_b{b}")
                nc.scalar.activation(out=sd[:G], in_=var[:G], func=AF.Sqrt,
                                     bias=eps_sb[:G])
                nc.vector.reciprocal(out=gmr[:G, 1:2], in_=sd[:G])
                st_["gmr", b] = gmr

            def normalize(b):
                x_sb, gmr = st_["x", b], st_.pop(("gmr", b))
                n_sb = []
                for c in range(NCH):
                    bc = pp.tile([P, 2], F32, tag="sp", name=f"bc_b{b}_{c}")
                    nc.tensor.matmul(bc, lhsT=gmT_sb[:, c * P:(c + 1) * P],
                                     rhs=gmr, start=True, stop=True)
                    a = wpool.tile([P, 1], F32, tag=f"a{c}", name=f"a_b{b}_{c}")
                    nc.vector.tensor_mul(a, bc[:, 1:2], bias_sb[c][:, 2:3])
                    gt = wpool.tile([P, 1], F32, tag=f"gt{c}", name=f"gt_b{b}_{c}")
                    nc.vector.tensor_mul(gt, bc[:, 0:1], a)
                    bb = wpool.tile([P, 1], F32, tag=f"bb{c}", name=f"bb_b{b}_{c}")
                    nc.vector.tensor_sub(bb, bias_sb[c][:, 3:4], gt)
                    nt = wpool.tile([P, HW], F32R, tag=f"n{c}", name=f"n_b{b}_{c}")
                    nc.vector.tensor_scalar(nt, x_sb[c], a, bb, OP.mult, OP.add)
                    n_sb.append(nt)
                st_["n", b] = n_sb

            def qkv(b):
                n_sb = st_.pop(("n", b))
                # q: + bq (DVE per-partition add); k: bias cancels -> ACT copy
                for (w_t, tagbase) in ((wq_sb, "q"), (wk_sb, "k")):
                    dst = []
                    for o in range(NCH):
                        acc = pp.tile([P, HW], F32, tag="acc2",
                                      name=f"{tagbase}acc_b{b}_{o}")
                        for c in range(NCH):
                            for h in range(2):
                                nc.tensor.matmul(
                                    acc[:, h * HWH:(h + 1) * HWH],
                                    lhsT=w_t[c][:, o * P:(o + 1) * P],
                                    rhs=n_sb[c][:, h * HWH:(h + 1) * HWH],
                                    start=(c == 0), stop=(c == NCH - 1))
                        t = wpool.tile([P, HW], F32R, tag=f"{tagbase}{o}",
                                       name=f"{tagbase}_b{b}_{o}")
                        if tagbase == "q":
                            nc.vector.tensor_scalar(t, acc,
                                                    bias_sb[o][:, 0:1],
                                                    None, OP.add)
                        else:
                            nc.scalar.copy(t, acc)
                        dst.append(t)
                    st_[tagbase, b] = dst
                v_sb = []
                for t8 in range(NPT):
                    acc = pp.tile([P, HWH], F32, tag="acc1", name=f"vacc_b{b}_{t8}")
                    for c in range(NCH):
                        nc.tensor.matmul(acc,
                                         lhsT=n_sb[c][:, t8 * P:(t8 + 1) * P],
                                         rhs=wv_sb[c],
                                         start=(c == 0), stop=(c == NCH - 1))
                    vt = wpool.tile([P, HWH], F32R, tag=f"v{t8}",
                                    name=f"v_b{b}_{t8}")
                    nc.scalar.copy(vt, acc)
                    v_sb.append(vt)
                st_["v", b] = v_sb

            def st_phase(b):
                q_sb, k_sb = st_.pop(("q", b)), st_.pop(("k", b))
                e_sb = []
                for m in range(NPT):
                    acc = pp.tile([P, HW], F32, tag="acc2", name=f"sacc_b{b}_{m}")
                    for c in range(NCH):
                        for h in range(2):
                            nc.tensor.matmul(
                                acc[:, h * HWH:(h + 1) * HWH],
                                lhsT=k_sb[c][:, m * P:(m + 1) * P],
                                rhs=q_sb[c][:, h * HWH:(h + 1) * HWH],
                                start=(c == 0), stop=(c == NCH - 1))
                    et = wpool.tile([P, HW], F32R, tag=f"e{m}", name=f"e_b{b}_{m}")
                    nc.scalar.activation(out=et, in_=acc, func=AF.Exp,
                                         bias=zero_col, scale=SCALE)
                    e_sb.append(et)
                st_["e", b] = e_sb

            def den_phase(b):
                # denominator row + its reciprocal run on PE/DVE while the
                # AV matmuls (27us of PE work) execute - hides the slow
                # single-partition DVE reciprocal completely.
                e_sb = st_["e", b]
                rd = wpool.tile([1, HW], F32R, tag="rd", name=f"rd_b{b}")
                for h in range(2):
                    dn = pp.tile([1, HWH], F32, tag="sp", name=f"dn_b{b}_{h}")
                    for m in range(NPT):
                        nc.tensor.matmul(dn, lhsT=ones_col,
                                         rhs=e_sb[m][:, h * HWH:(h + 1) * HWH],
                                         start=(m == 0), stop=(m == NPT - 1))
                    with nc.allow_low_precision(reason="fp32r feed for PE bcast"):
                        nc.vector.reciprocal(out=rd[:, h * HWH:(h + 1) * HWH],
                                             in_=dn)
                st_["rd", b] = rd

            def av_den(b):
                e_sb, v_sb = st_["e", b], st_.pop(("v", b))
                o_sb = []
                for ct in range(NCH):
                    acc = pp.tile([P, HW], F32, tag="acc2", name=f"oacc_b{b}_{ct}")
                    for m in range(NPT):
                        for h in range(2):
                            nc.tensor.matmul(
                                acc[:, h * HWH:(h + 1) * HWH],
                                lhsT=v_sb[m][:, ct * P:(ct + 1) * P],
                                rhs=e_sb[m][:, h * HWH:(h + 1) * HWH],
                                start=(m == 0), stop=(m == NPT - 1))
                    ot = wpool.tile([P, HW], F32R, tag=f"q{ct}", name=f"o_b{b}_{ct}")
                    nc.scalar.copy(ot, acc)
                    o_sb.append(ot)
                st_["o", b] = o_sb

                rd = st_.pop(("rd", b))
                racc = pp.tile([P, HW], F32, tag="acc2", name=f"racc_b{b}")
                for h in range(2):
                    nc.tensor.matmul(racc[:, h * HWH:(h + 1) * HWH],
                                     lhsT=ones_row,
                                     rhs=rd[:1, h * HWH:(h + 1) * HWH],
                                     start=True, stop=True)
                r_sb = wpool.tile([P, HW], F32, tag="r", name=f"r_b{b}")
                nc.scalar.copy(r_sb, racc)
                st_.pop(("e", b))
                st_["r", b] = r_sb

            def proj(b):
                o_sb, r_sb = st_.pop(("o", b)), st_.pop(("r", b))
                x_sb = st_.pop(("x", b))
                for o in range(NCH):
                    acc = pp.tile([P, HW], F32, tag="acc2", name=f"pacc_b{b}_{o}")
                    for c in range(NCH):
                        for h in range(2):
                            nc.tensor.matmul(
                                acc[:, h * HWH:(h + 1) * HWH],
                                lhsT=wp_sb[c][:, o * P:(o + 1) * P],
                                rhs=o_sb[c][:, h * HWH:(h + 1) * HWH],
                                start=(c == 0), stop=(c == NCH - 1))
                    t1 = wpool.tile([P, HW], F32, tag="t1", bufs=2,
                                    name=f"t1_b{b}_{o}")
                    nc.vector.tensor_mul(t1, acc, r_sb)
                    yt = wpool.tile([P, HW], F32, tag=f"y{o}", name=f"y_b{b}_{o}")
                    nc.vector.scalar_tensor_tensor(
                        out=yt, in0=t1, scalar=bias_sb[o][:, 1:2], in1=x_sb[o],
                        op0=OP.add, op1=OP.add)
                    oeng = nc.sync if o % 2 == 0 else nc.gpsimd
                    oeng.dma_start(out=ys_ap[b, o * P:(o + 1) * P, :], in_=yt)

            # ---- software-pipelined emission, one image ahead ----
            gn_stats(0)
            normalize(0)
            qkv(0)
            for b in range(BPC):
                st_phase(b)
                den_phase(b)
                if b + 1 < BPC:
                    load_x(b + 1)
                    gn_stats(b + 1)
                av_den(b)
                if b + 1 < BPC:
                    normalize(b + 1)
                proj(b)
                if b + 1 < BPC:
                    qkv(b + 1)

    nc.compile()
    return nc


_NC = None


def _get_nc():
    global _NC
    if _NC is None:
        _NC = _build()
    return _NC


def _host_inputs(x, gn_scale, gn_bias, wq, bq, wk, bk, wv, bv, wp, bp):
    x = np.ascontiguousarray(np.asarray(x, np.float32).reshape(B, C, HW))
    f = lambda t: np.ascontiguousarray(np.asarray(t, np.float32))
    gn_scale, gn_bias = f(gn_scale), f(gn_bias)
    bq, bv, bp = f(bq), f(bv), f(bp)
    wq, wk, wv, wp = f(wq), f(wk), f(wv), f(wp)

    bp_eff = bp + wp @ bv  # v-bias passes through softmax-averaging intact
    biasp = np.stack([bq, bp_eff, gn_scale, gn_bias], 1).reshape(NCH, P, 4)
    ch = np.arange(C)
    gmask_full = (ch[:, None] // GS == np.arange(G)[None, :]).astype(np.float32)
    gmask = np.ascontiguousarray(gmask_full.reshape(NCH, P, G))
    gmaskT = np.zeros((P, C), np.float32)
    gmaskT[:G, :] = gmask_full.T
    common = {
        "wqT": np.ascontiguousarray(wq.T),
        "wkT": np.ascontiguousarray(wk.T),
        "wvT": np.ascontiguousarray(wv.T),
        "wpT": np.ascontiguousarray(wp.T),
        "biasp": np.ascontiguousarray(biasp),
        "gmask": gmask,
        "gmaskT": gmaskT,
        "onesd": np.ones((P, P), np.float32),
    }
    in_maps = []
    for i in range(NCORES):
        m = dict(common)
        m["xs"] = np.ascontiguousarray(x[i * BPC:(i + 1) * BPC])
        in_maps.append(m)
    return in_maps


def _run(in_maps, trace=False):
    nc = _get_nc()
    return bass_utils.run_bass_kernel_spmd(nc, in_maps, list(range(NCORES)),
                                           trace=trace)


def kernel(**inputs):
    in_maps = _host_inputs(**inputs)
    res = _run(in_maps, trace=False)
    y = np.concatenate([r["ys"] for r in res.results], axis=0)
    return y.reshape(B, C, H, W)


def run_traced(**inputs):
    """Like kernel() but with NTFF tracing; returns (y, exec_time_ns)."""
    in_maps = _host_inputs(**inputs)
    res = _run(in_maps, trace=True)
    y = np.concatenate([r["ys"] for r in res.results], axis=0)
    return y.reshape(B, C, H, W), res.exec_time_ns
